# revision 12
# baseline (speedup 1.0000x reference)
"""Trainium2 Bass kernel for nn_ConstrainedEnhancementModel.

Contract: kernel(**inputs) takes the FULL unsharded inputs (as produced by
reference.setup_inputs()) and returns the FULL [4096, 2000, 6] float32 output.

Strategy (pure data parallel over 8 NeuronCores, 512 batch rows each):
  - Feature-major MLP chain: every hidden activation is stored [feat, batch]
    so torch-layout weights [fan_in, fan_out] are directly the matmul lhsT.
  - x is uploaded pre-transposed into the window-blocked layout (host-side
    numpy), so the kernel has no PE transposes and reads x once in bf16.
  - The constraint/interpolation epilogue is folded into the final matmul:
        out = h5 @ (W6 * c_dec) + x @ G + ones * (b6 * c_dec)
    where G is a sparse constant [600, 12000] matrix holding the linear
    interpolation + anchor/blend coefficients.
  - Windows 0..23 (timesteps 0..1919) only ever see the decoded signal
    scaled by 0.2, so their W6*c_dec slice and h5 run in fp8e4m3 with
    DoubleRow matmuls (2 K-planes of 128 per instruction = 2x throughput).
    Window 24 contains the c_dec=1.0 tail and stays bf16.
  - Output is stored bf16 (halves the dominant HBM write) and upcast to
    f32 on the host; rel-err budget is 2e-2, measured total err ~5e-3.
"""

import numpy as np
import ml_dtypes

import bass_rust
import concourse.bass as bass
import concourse.bacc as bacc
import concourse.mybir as mybir
import concourse.tile as tile
from concourse import bass_utils

F32 = mybir.dt.float32
BF16 = mybir.dt.bfloat16
FP8 = mybir.dt.float8e4
BF16_NP = ml_dtypes.bfloat16
FP8_NP = ml_dtypes.float8_e4m3fn

# Problem config (hardcoded; must match the reference)
LOW_T = 100
HIGH_T = 2000
FEAT = 6
HID = 256
NUM_CLASSES = 10
LBL_DIM = 16
UP = 20
B = 4096
NCORES = 8
BC = B // NCORES          # 512 batch rows per core
NBT = BC // 128           # 4 batch tiles per core
D_IN = LOW_T * FEAT       # 600
D_OUT = HIGH_T * FEAT     # 12000
NW = 25                   # output windows (80 timesteps * 6 feats = 480 cols)
WT = 480
NI4 = 7                   # ceil(25/4) groups of 4 windows; blocks 0..5 full


def _ap3(t_ap, off, d1, n1, d2, n2):
    """Build a 3D [partition, n1, n2] AP from a tile's 2D AP."""
    return bass_rust.AP(
        tensor=t_ap.tensor, offset=t_ap.offset + off,
        ap=[[t_ap.ap[0][0], t_ap.ap[0][1]], [d1, n1], [d2, n2]],
    )


def _build_nc():
    """Build the single-core Bass program (SPMD: same program on all 8)."""
    nc = bacc.Bacc("TRN2", target_bir_lowering=False, debug=False)

    xre_d = nc.dram_tensor("xre", [128, NI4 * 512], BF16, kind="ExternalInput")
    lab_d = nc.dram_tensor("labf", [1, BC], BF16, kind="ExternalInput")
    w1_d = nc.dram_tensor("w1re", [NI4, 128, 512], BF16, kind="ExternalInput")
    w2_d = nc.dram_tensor("w2", [512, 256], BF16, kind="ExternalInput")
    w3_d = nc.dram_tensor("w3", [256, 128], BF16, kind="ExternalInput")
    w4a_d = nc.dram_tensor("w4a", [128, 256], BF16, kind="ExternalInput")
    w4b_d = nc.dram_tensor("w4b", [16, 256], BF16, kind="ExternalInput")
    w5_d = nc.dram_tensor("w5", [256, 512], BF16, kind="ExternalInput")
    # fp8 W6*c_d, blocked: [i4][p][w][j][plane][480] with K-chunk = 2j+plane
    w6q_d = nc.dram_tensor("w6q", [6, 128, 16 * WT], FP8, kind="ExternalInput")
    # bf16 W6*c_d for window 24: [p][k][480]
    w6t_d = nc.dram_tensor("w6t", [128, 4 * WT], BF16, kind="ExternalInput")
    # all biases packed: cols 0-3 b1, 4-5 b2, 6 b3, 7-8 b4, 9-12 b5
    ball_d = nc.dram_tensor("ball", [128, 16], F32, kind="ExternalInput")
    emb_d = nc.dram_tensor("embT", [NUM_CLASSES, LBL_DIM], BF16, kind="ExternalInput")
    iota_d = nc.dram_tensor("iota10", [NUM_CLASSES, 1], F32, kind="ExternalInput")
    g_d = nc.dram_tensor("gmat", [128, NI4 * WT], BF16, kind="ExternalInput")
    y_d = nc.dram_tensor("y", [BC, D_OUT], BF16, kind="ExternalOutput")

    RELU = mybir.ActivationFunctionType.Relu
    DR = mybir.MatmulPerfMode.DoubleRow

    with tile.TileContext(nc) as tc:
        with (
            tc.tile_pool(name="const", bufs=1) as cp,
            tc.tile_pool(name="outpool", bufs=4) as op,
            tc.tile_pool(name="ppool", bufs=8, space="PSUM") as pm,
        ):
            # ---- persistent SBUF tensors ----
            cw1 = [cp.tile([128, 512], BF16, tag=f"cw1_{i}", name=f"cw1_{i}") for i in range(NI4)]
            cw2 = [cp.tile([128, 256], BF16, tag=f"cw2_{i}", name=f"cw2_{i}") for i in range(4)]
            cw3 = [cp.tile([128, 128], BF16, tag=f"cw3_{i}", name=f"cw3_{i}") for i in range(2)]
            cw4a = cp.tile([128, 256], BF16, tag="cw4a", name="cw4a")
            cw4b = cp.tile([16, 256], BF16, tag="cw4b", name="cw4b")
            cw5 = [cp.tile([128, 512], BF16, tag=f"cw5_{i}", name=f"cw5_{i}") for i in range(2)]
            cw6t = cp.tile([128, 4 * WT], BF16, tag="cw6t", name="cw6t")
            cw6q = [cp.tile([128, 16 * WT], FP8, tag=f"cw6q_{i}", name=f"cw6q_{i}") for i in range(6)]
            cball = cp.tile([128, 16], F32, tag="cball", name="cball")
            cb1 = [cball[:, i:i + 1] for i in range(4)]
            cb2 = [cball[:, 4 + i:5 + i] for i in range(2)]
            cb3 = cball[:, 6:7]
            cb4 = [cball[:, 7 + i:8 + i] for i in range(2)]
            cb5 = [cball[:, 9 + i:10 + i] for i in range(4)]
            warm = cp.tile([1, 16], BF16, tag="warm", name="warm")
            cemb = cp.tile([NUM_CLASSES, LBL_DIM], BF16, tag="cemb", name="cemb")
            ciota = cp.tile([NUM_CLASSES, 1], F32, tag="ciota", name="ciota")
            cg = cp.tile([128, NI4 * WT], BF16, tag="cg", name="cg")
            clab = cp.tile([1, BC], BF16, tag="clab", name="clab")
            ones10 = cp.tile([1, NUM_CLASSES], BF16, tag="ones10", name="ones10")
            xre = cp.tile([128, NI4 * 512], BF16, tag="xre", name="xre")
            h1 = [cp.tile([128, BC], BF16, tag=f"h1_{i}", name=f"h1_{i}") for i in range(4)]
            h2 = [cp.tile([128, BC], BF16, tag=f"h2_{i}", name=f"h2_{i}") for i in range(2)]
            feat = cp.tile([128, BC], BF16, tag="feat", name="feat")
            h4 = [cp.tile([128, BC], BF16, tag=f"h4_{i}", name=f"h4_{i}") for i in range(2)]
            h5 = [cp.tile([128, BC], BF16, tag=f"h5_{i}", name=f"h5_{i}") for i in range(4)]
            # fp8 h5, K-plane interleaved: h5q[j][:, plane*BC + b] = h5 chunk 2j+plane
            h5q = [cp.tile([128, 2 * BC], FP8, tag=f"h5q_{j}", name=f"h5q_{j}") for j in range(2)]
            onehot = cp.tile([NUM_CLASSES, BC], BF16, tag="onehot", name="onehot")
            embt = cp.tile([LBL_DIM, BC], BF16, tag="embt", name="embt")

            # warm up the scalar engine's activation table at t=0 so the
            # first real activation doesn't eat the 1.3us ACT_TABLE_LOAD
            nc.gpsimd.memset(ones10[:], 1.0)
            nc.scalar.activation(warm[:, 0:NUM_CLASSES], ones10[:], RELU)

            # ---- const loads ----
            # tiny PE-gating transfers first (the label matmuls stall on
            # these), then xre + w1re (gate L1), with the six w6q blocks
            # interleaved so they stream behind what the encoder needs
            nc.sync.dma_start(clab[:], lab_d[:])
            nc.sync.dma_start(ciota[:], iota_d[:])
            nc.sync.dma_start(cemb[:], emb_d[:])
            nc.sync.dma_start(cball[:], ball_d[:])
            for i in range(NI4):
                nc.sync.dma_start(xre[:, i * 512:(i + 1) * 512],
                                  xre_d[:, i * 512:(i + 1) * 512])
                nc.sync.dma_start(cw1[i][:], w1_d[i])
            nc.sync.dma_start(cw6q[0][:], w6q_d[0])
            for k in range(4):
                nc.sync.dma_start(cw2[k][:], w2_d[k * 128:(k + 1) * 128, :])
            for k in range(2):
                nc.sync.dma_start(cw3[k][:], w3_d[k * 128:(k + 1) * 128, :])
            nc.sync.dma_start(cw4a[:], w4a_d[:])
            nc.sync.dma_start(cw4b[:], w4b_d[:])
            for k in range(2):
                nc.sync.dma_start(cw5[k][:], w5_d[k * 128:(k + 1) * 128, :])
            nc.sync.dma_start(cw6q[1][:], w6q_d[1])
            nc.sync.dma_start(cg[:], g_d[:])
            nc.sync.dma_start(cw6t[:], w6t_d[:])
            for i in range(2, 6):
                nc.sync.dma_start(cw6q[i][:], w6q_d[i])

            # ---- label one-hot + embedding (feature-major [16, BC]) ----
            psl = pm.tile([128, 512], F32, tag="ps", name="ps")
            nc.tensor.matmul(psl[0:NUM_CLASSES, 0:BC], ones10[:], clab[:],
                             start=True, stop=True)
            nc.vector.tensor_scalar(
                onehot[:], psl[0:NUM_CLASSES, 0:BC], ciota[:], None,
                mybir.AluOpType.is_equal,
            )
            pse = pm.tile([128, 512], F32, tag="ps", name="ps")
            nc.tensor.matmul(pse[0:LBL_DIM, 0:BC], cemb[:], onehot[:],
                             start=True, stop=True)
            nc.vector.tensor_copy(embt[:], pse[0:LBL_DIM, 0:BC])

            # ---- encoder / decoder MLP (feature-major, N = BC) ----
            # L1: [600->512] via window-blocked x / rearranged W1.
            # i4-outer so the first matmuls only wait on xre/cw1 block 0's
            # DMA and later blocks stream in behind the compute.
            psl1 = [pm.tile([128, 512], F32, tag="ps", name="ps") for _ in range(4)]
            for i4 in range(NI4):
                for m in range(4):
                    nc.tensor.matmul(
                        psl1[m][:, 0:BC], cw1[i4][:, m * 128:(m + 1) * 128],
                        xre[:, i4 * 512:(i4 + 1) * 512],
                        start=(i4 == 0), stop=(i4 == NI4 - 1),
                    )
            for m in range(4):
                if m % 2 == 0:
                    nc.scalar.activation(h1[m][:], psl1[m][:, 0:BC], RELU, bias=cb1[m][:])
                else:
                    nc.vector.tensor_scalar(h1[m][:], psl1[m][:, 0:BC], cb1[m][:], 0.0, mybir.AluOpType.add, mybir.AluOpType.max)
            # L2: [512->256]
            for m in range(2):
                ps = pm.tile([128, 512], F32, tag="ps", name="ps")
                for k in range(4):
                    nc.tensor.matmul(
                        ps[:, 0:BC], cw2[k][:, m * 128:(m + 1) * 128], h1[k][:],
                        start=(k == 0), stop=(k == 3),
                    )
                if m % 2 == 0:
                    nc.scalar.activation(h2[m][:], ps[:, 0:BC], RELU, bias=cb2[m][:])
                else:
                    nc.vector.tensor_scalar(h2[m][:], ps[:, 0:BC], cb2[m][:], 0.0, mybir.AluOpType.add, mybir.AluOpType.max)
            # L3: [256->128], no relu
            ps = pm.tile([128, 512], F32, tag="ps", name="ps")
            for k in range(2):
                nc.tensor.matmul(ps[:, 0:BC], cw3[k][:], h2[k][:],
                                 start=(k == 0), stop=(k == 1))
            nc.vector.tensor_scalar(feat[:], ps[:, 0:BC], cb3[:], None, mybir.AluOpType.add)
            # L4: [144->256] = feat part + label-embedding part
            for m in range(2):
                ps = pm.tile([128, 512], F32, tag="ps", name="ps")
                nc.tensor.matmul(ps[:, 0:BC], cw4a[:, m * 128:(m + 1) * 128],
                                 feat[:], start=True, stop=False)
                nc.tensor.matmul(ps[:, 0:BC], cw4b[:, m * 128:(m + 1) * 128],
                                 embt[:], start=False, stop=True)
                if m % 2 == 0:
                    nc.scalar.activation(h4[m][:], ps[:, 0:BC], RELU, bias=cb4[m][:])
                else:
                    nc.vector.tensor_scalar(h4[m][:], ps[:, 0:BC], cb4[m][:], 0.0, mybir.AluOpType.add, mybir.AluOpType.max)
            # L5: [256->512]; also cast each chunk to fp8 (plane-interleaved)
            for m in range(4):
                ps = pm.tile([128, 512], F32, tag="ps", name="ps")
                for k in range(2):
                    nc.tensor.matmul(
                        ps[:, 0:BC], cw5[k][:, m * 128:(m + 1) * 128], h4[k][:],
                        start=(k == 0), stop=(k == 1),
                    )
                # both the bf16 and fp8 copies read the psum directly so
                # neither is serialized behind the other
                dstq = h5q[m // 2][:, (m % 2) * BC:(m % 2) * BC + BC]
                if m % 2 == 0:
                    nc.scalar.activation(dstq, ps[:, 0:BC], RELU, bias=cb5[m][:])
                    nc.vector.tensor_scalar(h5[m][:], ps[:, 0:BC], cb5[m][:], 0.0, mybir.AluOpType.add, mybir.AluOpType.max)
                else:
                    nc.vector.tensor_scalar(dstq, ps[:, 0:BC], cb5[m][:], 0.0, mybir.AluOpType.add, mybir.AluOpType.max)
                    nc.scalar.activation(h5[m][:], ps[:, 0:BC], RELU, bias=cb5[m][:])

            # ---- final layer + fused constraint epilogue ----
            # Blocks 0..5: fp8 DoubleRow, 2 instructions per window (K=512 as
            # 2x2 planes of 128), then the four K=32 G matmuls back-to-back
            # on distinct PE row groups / psum banks (they run concurrently).
            for i4 in range(6):
                w6b = cw6q[i4]
                for bt in range(NBT):
                    pss = []
                    for w in range(4):
                        ps = pm.tile([128, 512], F32, tag="ps", name="ps")[:, 0:WT]
                        pss.append(ps)
                    for j in range(2):
                        lhs = _ap3(h5q[j][:], bt * 128, BC, 2, 1, 128)
                        for w in range(4):
                            rhs = _ap3(w6b[:], (2 * w + j) * 2 * WT, WT, 2, 1, WT)
                            nc.tensor.matmul(
                                pss[w][:], lhs, rhs,
                                start=(j == 0), stop=False, perf_mode=DR,
                            )
                    for w in range(4):
                        p0 = 32 * w
                        nc.tensor.matmul(
                            pss[w][:],
                            xre[p0:p0 + 32, i4 * 512 + bt * 128:i4 * 512 + (bt + 1) * 128],
                            cg[p0:p0 + 32, i4 * WT:(i4 + 1) * WT],
                            start=False, stop=True, tile_position=(p0, 0),
                        )
                    ob = op.tile([128, 4 * WT], BF16, tag="ob", name="ob")
                    for w in range(4):
                        if w % 2 == 0:
                            nc.vector.tensor_copy(ob[:, w * WT:(w + 1) * WT], pss[w][:])
                        else:
                            nc.scalar.copy(ob[:, w * WT:(w + 1) * WT], pss[w][:])
                    nc.sync.dma_start(
                        y_d[bt * 128:(bt + 1) * 128, i4 * 4 * WT:(i4 + 1) * 4 * WT],
                        ob[:],
                    )

            # Block 6: window 24 only (timesteps 1920..1999 incl. the c_d=1.0
            # tail) in bf16.
            for bt in range(NBT):
                ps = pm.tile([128, 512], F32, tag="ps", name="ps")[:, 0:WT]
                for k in range(4):
                    nc.tensor.matmul(
                        ps[:], h5[k][:, bt * 128:(bt + 1) * 128],
                        cw6t[:, k * WT:(k + 1) * WT],
                        start=(k == 0), stop=False,
                    )
                nc.tensor.matmul(
                    ps[:],
                    xre[0:32, 6 * 512 + bt * 128:6 * 512 + (bt + 1) * 128],
                    cg[0:32, 6 * WT:7 * WT],
                    start=False, stop=True, tile_position=(0, 0),
                )
                ob = op.tile([128, WT], BF16, tag="ob6", name="ob6")
                if bt % 2 == 0:
                    nc.vector.tensor_copy(ob[:], ps[:])
                else:
                    nc.scalar.copy(ob[:], ps[:])
                nc.sync.dma_start(
                    y_d[bt * 128:(bt + 1) * 128, 24 * WT:25 * WT], ob[:]
                )

    nc.compile()
    return nc


def _blend_coeffs():
    """Per-timestep blend coefficients (match the reference formulas)."""
    t = np.arange(HIGH_T)
    seg = np.clip(t // UP, 0, LOW_T - 2)
    alpha = ((t - seg * UP) / UP).astype(np.float64)
    is_anchor = (t % UP) == 0
    interior = t < (LOW_T - 1) * UP
    blendf = np.where(is_anchor, 1.0, np.where(interior, 0.8, 0.0))
    c_d = np.where(is_anchor, 0.0, np.where(interior, 0.2, 1.0))
    c_start = blendf * (1.0 - alpha)
    c_end = blendf * alpha
    return seg, c_d, c_start, c_end


def _host_prep(inputs):
    """Build per-core in_maps from the full inputs."""
    x_full = np.asarray(inputs["low_res_data"], np.float32).reshape(B, D_IN)
    labels = np.asarray(inputs["labels"]).astype(np.float32)
    W1 = np.asarray(inputs["W1"], np.float32)
    W6 = np.asarray(inputs["W6"], np.float32)
    b6 = np.asarray(inputs["b6"], np.float32)

    seg, c_d, c_start, c_end = _blend_coeffs()

    # G matrix, window-blocked: [128, NI4*480]; window i lives at partition
    # offset 32*(i%4), col block i//4.  Rows r=0..29 <-> x col 24*i + r,
    # row 30 = bias row (paired with the constant-1.0 row of xre).
    gmat = np.zeros((128, NI4 * WT), np.float64)
    for tt in range(HIGH_T):
        i, dt = divmod(tt, 80)
        i4, wpos = divmod(i, 4)
        p0 = 32 * wpos
        sl = seg[tt] - 4 * i
        for f in range(FEAT):
            col = i4 * WT + FEAT * dt + f
            gmat[p0 + FEAT * sl + f, col] += c_start[tt]
            gmat[p0 + FEAT * (sl + 1) + f, col] += c_end[tt]
            gmat[p0 + 30, col] = c_d[tt] * np.float64(b6[FEAT * tt + f])
    gmat = gmat.astype(np.float32).astype(BF16_NP)

    c_d_full = np.repeat(c_d, FEAT).astype(np.float32)
    w6c = W6 * c_d_full[None, :]                     # [512, 12000]
    # fp8 blocked layout for windows 0..23: [6][128][w][j][plane][480]
    # K-chunk index = 2j+plane; W6 row = 128*(2j+plane)+p.
    w6b = w6c[:, :24 * WT].reshape(512, 6, 4, WT)    # [k*128+p, i4, w, c]
    w6b = w6b.reshape(2, 2, 128, 6, 4, WT)           # [j, plane, p, i4, w, c]
    w6q = np.ascontiguousarray(
        w6b.transpose(3, 2, 4, 0, 1, 5)              # [i4, p, w, j, plane, c]
    ).reshape(6, 128, 16 * WT).astype(FP8_NP)
    # bf16 for window 24: [p][k][480]
    w6t = np.ascontiguousarray(
        w6c[:, 24 * WT:].reshape(4, 128, WT).transpose(1, 0, 2)
    ).reshape(128, 4 * WT).astype(BF16_NP)

    # W1 rearranged to the window-blocked x layout (duplicated/ones/pad rows
    # get zero weights)
    w1re = np.zeros((NI4, 128, 512), np.float32)
    for c in range(D_IN):
        i, r = divmod(c, 24)
        i4, wpos = divmod(i, 4)
        w1re[i4, 32 * wpos + r, :] = W1[c, :]
    w1re = w1re.astype(BF16_NP)

    # packed biases [128, 16]: cols 0-3 b1, 4-5 b2, 6 b3, 7-8 b4, 9-12 b5
    ball = np.zeros((128, 16), np.float32)
    ball[:, 0:4] = np.asarray(inputs["b1"], np.float32).reshape(4, 128).T
    ball[:, 4:6] = np.asarray(inputs["b2"], np.float32).reshape(2, 128).T
    ball[:, 6] = np.asarray(inputs["b3"], np.float32)
    ball[:, 7:9] = np.asarray(inputs["b4"], np.float32).reshape(2, 128).T
    ball[:, 9:13] = np.asarray(inputs["b5"], np.float32).reshape(4, 128).T

    const_map = {
        "w1re": w1re,
        "w2": np.asarray(inputs["W2"], np.float32).astype(BF16_NP),
        "w3": np.asarray(inputs["W3"], np.float32).astype(BF16_NP),
        "w4a": np.asarray(inputs["W4"], np.float32)[:128].astype(BF16_NP),
        "w4b": np.asarray(inputs["W4"], np.float32)[128:144].astype(BF16_NP),
        "w5": np.asarray(inputs["W5"], np.float32).astype(BF16_NP),
        "w6q": w6q,
        "w6t": w6t,
        "ball": ball,
        "embT": np.asarray(inputs["emb"], np.float32).astype(BF16_NP),
        "iota10": np.arange(NUM_CLASSES, dtype=np.float32).reshape(NUM_CLASSES, 1),
        "gmat": gmat,
    }

    # window-blocked transposed x per core: xre[32*w + r, i4*512 + b] =
    # x[b, 24*(4*i4+w) + r] for r<30; row 32*w+30 = 1.0 (G bias), 31 = 0.
    xw = x_full.reshape(B, 25, 24)                   # [b, i, r] r=0..23
    # window i needs x cols 24i..24i+30 => rows r=0..29: cols 24i+r; for
    # r>=24 that's the first 6 cols of window i+1.
    xr30 = np.zeros((B, 25, 30), np.float32)
    xr30[:, :, :24] = xw
    xr30[:, :24, 24:] = xw[:, 1:, :6]
    # last window (i=24) rows 24..29 correspond to x cols 600..605 (absent);
    # G never references them (segment 98 is the last), leave zero.

    in_maps = []
    for c in range(NCORES):
        sl = slice(c * BC, (c + 1) * BC)
        xc = xr30[sl]                                # [BC, 25, 30]
        xre = np.zeros((128, NI4 * 512), np.float32)
        for i4 in range(NI4):
            nwin = 4 if i4 < 6 else 1
            for w in range(nwin):
                i = 4 * i4 + w
                xre[32 * w:32 * w + 30, i4 * 512:(i4 + 1) * 512] = \
                    xc[:, i, :].T
            xre[32 * np.arange(4) + 30, i4 * 512:(i4 + 1) * 512] = 1.0
        m = dict(const_map)
        m["xre"] = xre.astype(BF16_NP)
        m["labf"] = labels[sl].reshape(1, BC).astype(BF16_NP)
        in_maps.append(m)
    return in_maps


_NC_CACHE = None


def kernel(**inputs) -> np.ndarray:
    global _NC_CACHE
    if _NC_CACHE is None:
        _NC_CACHE = _build_nc()
    nc = _NC_CACHE
    in_maps = _host_prep(inputs)
    res = bass_utils.run_bass_kernel_spmd(nc, in_maps, core_ids=list(range(NCORES)))
    out = np.concatenate(
        [np.asarray(res.results[c]["y"]).astype(np.float32) for c in range(NCORES)],
        axis=0,
    )
    return out.reshape(B, HIGH_T, FEAT)
